# revision 43
# baseline (speedup 1.0000x reference)
"""AttentionRNNCell (streaming-softmax attention RNN) for 8 TRN2 NeuronCores.

kernel(x, kv_kernel, q_kernel) -> [B, T, D] float32

Math per (batch, head): kv = silu(x @ kv_kernel); s_t = <q_h, k_t>;
out_t = sum_h cumsum_t(v * e^s) / cumsum_t(e^s)   (unstabilized streaming
softmax — safe for this data distribution; |s| stays < ~8).

Strategy (data-parallel over batch, 4 batches/core):
  - Projections run in fp8e4m3 DoubleRow mode (2 fp8 weights/cell, 256-deep
    contraction): K path fully fp8 (its quantization noise only perturbs
    softmax weights, which streaming averaging washes out), V path fp8 for
    t>=128 with the first chunk bf16 (at small t, h_t averages few v's, so
    V-path fp8 noise passes straight through).
  - K^T = Wk^T @ x^T on PE in [head*dim, t] layout (f32 psum), silu on ACT
    straight out of PSUM, s^T = Qblock^T @ silu(K^T) on PE, exp on ACT,
    den = cumsum via DVE tensor_tensor_scan along t, PE-transposes bring
    e/1/den back to [t, h] layout.  The whole exp/scan/transpose/recip
    chain for batch b+1 is emitted as closures interleaved into batch b's
    chunk loop (together with b+1's K groups) so the PE never stalls at a
    batch boundary (HAM re-throttles after ~3.4us of idle).
  - V columns are permuted to (d, h) order on the host so the final
    head-sum reduce is unit-stride and the e/rden broadcasts are
    inner-contiguous (DVE 2x mode for the 16-bit ve multiply).
  - V = x^T.T @ Wv in [t, (d h)] layout, silu to bf16, ve = v*e (DVE bf16
    broadcast multiply), cumsum over t via a column-rotated triangular-ones
    matmul (output row 0 = running total -> legal K=1 carry-broadcast
    source for the next chunk), psum->sbuf cum copies split ACT/DVE,
    out = sum_h num * (1/den) via DVE multiply (bf16 out) + contiguous
    reduce into a per-batch staging tile, stored with 2 large DMAs.
    Rotation is undone on the host (free np.roll).
"""

import numpy as np
from contextlib import ExitStack

import ml_dtypes

import bass_rust
import concourse.bass as bass
import concourse.mybir as mybir
import concourse.tile as tile
from concourse import bass_utils

AF = mybir.ActivationFunctionType
BF16 = mybir.dt.bfloat16
FP8 = mybir.dt.float8e4
F32 = mybir.dt.float32
F32R = mybir.dt.float32r
DR = mybir.MatmulPerfMode.DoubleRow

P = 128
N_CORES = 8
B, T, I_DIM, H, D = 32, 1024, 1024, 16, 64
B_LOC = B // N_CORES


# ---------------------------------------------------------------------------
# TileContext patches: the walrus build in this container supports only ONE
# semaphore wait per instruction. (1) split the end-of-context drain's waits
# across several drains; (2) hoist extra scheduler-attached waits onto
# InstNoOp carriers just before the instruction on the same engine.
# ---------------------------------------------------------------------------

def _split_waits(self, inst):
    si = inst.sync_info
    if (
        si is not None
        and si.on_wait
        and len(si.on_wait) > 1
        and inst.engine != mybir.EngineType.Unassigned
    ):
        waits = list(si.on_wait)
        sem_waits = [w for w in waits if w.sync_type == "semaphore"]
        other = [w for w in waits if w.sync_type != "semaphore"]
        hoist = sem_waits[:-1] if sem_waits else []
        keep = sem_waits[-1:] + other if sem_waits else other
        if hoist:
            for w in hoist:
                nop = mybir.InstNoOp(
                    name=self.nc.get_next_instruction_name(),
                    sync_info=mybir.SyncInfo(on_wait=[w], on_update=[]),
                    bass_nofuse=True,
                    engine=inst.engine,
                )
                self.nc.register_instruction(nop, overwrite=True)
                self.nc.cur_bb.bb.add_instruction(nop)
            inst.sync_info = mybir.SyncInfo(
                on_wait=keep, on_update=list(si.on_update or [])
            )


def _patched_add_instruction(self, inst):
    _split_waits(self, inst)
    self.nc.register_instruction(inst, overwrite=True)
    self.nc.cur_bb.bb.add_instruction(inst)


def _patched_drain_and_barrier(self, tick_clock, wait_clock):
    nc = self.nc
    drain_inst = nc.sync.drain()
    wait_clock.add_sem_waits(
        drain_inst.ins, bass_rust.ScopedClock({None: tick_clock.global_clock})
    )
    si = drain_inst.ins.sync_info
    waits = list(si.on_wait) if si is not None and si.on_wait else []
    if len(waits) > 1:
        upds = list(si.on_update) if si.on_update else []
        drain_inst.ins.sync_info = bass_rust.SyncInfo(
            on_wait=[waits[0]], on_update=upds
        )
        for w in waits[1:]:
            extra = nc.sync.drain()
            extra.ins.sync_info = bass_rust.SyncInfo(on_wait=[w], on_update=[])

    nc.all_engine_barrier()
    assert self.sems is not None
    popped = nc._tile_sem_poison_stack.pop()
    assert popped is self._sem_poison
    nc.clear_and_free_semaphores(list(self.sems.allocated().values()))
    nc.all_engine_barrier()


def _apply_tile_patches():
    tile.TileContext._add_instruction = _patched_add_instruction
    tile.TileContext._drain_and_barrier = _patched_drain_and_barrier


# ---------------------------------------------------------------------------
# Kernel builder
# ---------------------------------------------------------------------------

def _build(nc, tc, ctx):
    B_loc, T_, I, H_, D_ = B_LOC, T, I_DIM, H, D
    HD = H_ * D_
    NT = T_ // P
    KT = I // P
    KP = KT // 2  # fp8 pair-blocks of 256 along I
    NG = HD // P
    NB = HD // 512
    TC5 = T_ // 512

    # bf16 inputs cover only the accuracy-critical early timesteps of the
    # V path (t<128); the K path runs fully in fp8 (its quantization noise
    # only perturbs softmax weights, which the streaming average washes out).
    xtb_d = nc.dram_tensor("xtb", [B_loc, I, P], BF16, kind="ExternalInput").ap()
    # fp8 x^T packed as I-block pairs: [b, j, p, s, t] = x^T[b, j*256+s*128+p, t]
    xt8_d = nc.dram_tensor("xt8", [B_loc, KP, P, 2, T_], FP8, kind="ExternalInput").ap()
    wk8_d = nc.dram_tensor("wk8", [KP, P, 2, HD], FP8, kind="ExternalInput").ap()
    wv_d = nc.dram_tensor("wv", [I, HD], BF16, kind="ExternalInput").ap()
    wv8_d = nc.dram_tensor("wv8", [KP, P, 2, HD], FP8, kind="ExternalInput").ap()
    qb8_d = nc.dram_tensor("qb8", [KP, P, 2, H_], FP8, kind="ExternalInput").ap()
    u_d = nc.dram_tensor("u", [P, P], BF16, kind="ExternalInput").ap()
    ones_d = nc.dram_tensor("ones", [1, P], F32R, kind="ExternalInput").ap()
    id_d = nc.dram_tensor("ident", [P, P], BF16, kind="ExternalInput").ap()
    # rotated rows: row m holds t=c*128+m-1 (row 0: t=c*128+127); host un-rolls.
    out_d = nc.dram_tensor("out", [B_loc, P, NT, D_], F32, kind="ExternalOutput").ap()

    const = ctx.enter_context(tc.tile_pool(name="const", bufs=1))
    xt_pool = ctx.enter_context(tc.tile_pool(name="xt", bufs=3 * KT))
    xt8_pool = ctx.enter_context(tc.tile_pool(name="xt8", bufs=3 * KP))
    ksil_pool = ctx.enter_context(tc.tile_pool(name="ksil", bufs=2))
    st_pool = ctx.enter_context(tc.tile_pool(name="st", bufs=6))
    epc_pool = ctx.enter_context(tc.tile_pool(name="epc", bufs=3 * NT))
    rden_pool = ctx.enter_context(tc.tile_pool(name="rden", bufs=4))
    vsil_pool = ctx.enter_context(tc.tile_pool(name="vsil", bufs=4))
    ve_pool = ctx.enter_context(tc.tile_pool(name="ve", bufs=2))
    cum_pool = ctx.enter_context(tc.tile_pool(name="cum", bufs=3))
    prod_pool = ctx.enter_context(tc.tile_pool(name="prod", bufs=2))
    o_pool = ctx.enter_context(tc.tile_pool(name="o", bufs=2))

    # PSUM: 8 banks. pa (3, shared tag) = K-path accumulators + transposes;
    # pv/pc 2 each -> 7 banks. (8/8 deadlocks the slot scheduler.)
    pa_pool = ctx.enter_context(tc.tile_pool(name="pa", bufs=3, space="PSUM"))
    pv_pool = ctx.enter_context(tc.tile_pool(name="pv", bufs=2, space="PSUM"))
    pc_pool = ctx.enter_context(tc.tile_pool(name="pc", bufs=2, space="PSUM"))

    # ---- weights/constants. Two DMA rings: sync carries weights, gpsimd
    # carries batch-0 activations, so the first K group's operands land in
    # parallel instead of serially on one ring. qb[0] goes first so the
    # act-table warm-up (below) starts immediately. ----
    qb8_sb = []
    for j in range(KP):
        t3 = const.tile([P, 2, H_], FP8, tag=f"qb8{j}")
        nc.sync.dma_start(t3[:], qb8_d[j, :, :, :])
        qb8_sb.append(t3)
    # dummy activation: pulls the one-time Silu act-table load (~2.7us,
    # measured as a PE stall before the first K silu) into the startup DMA
    # window where PE is idle anyway.
    warm = const.tile([P, H_], F32, tag="warm")
    nc.scalar.activation(warm[:], qb8_sb[0][:, 0, :], AF.Silu)

    wk8_sb, wv_sb, wv8_sb = [], [], []
    xt_b0, xt8_b0 = [], []
    for j in range(KP):
        t4 = const.tile([P, 2, HD], FP8, tag=f"wk8{j}")
        wk8_sb.append(t4)
        t = xt8_pool.tile([P, 2, T_], FP8, tag="xt8")
        nc.gpsimd.dma_start(t[:, :, 0:512], xt8_d[0, j, :, :, 0:512])
        xt8_b0.append(t)
    # wk8 lands g-major so the first K psum-group's weights (g=0 slices of
    # all 4 j) arrive in ~4 small DMAs instead of after the full 1MB load.
    for g in range(NG):
        for j in range(KP):
            nc.sync.dma_start(
                wk8_sb[j][:, :, g * P:(g + 1) * P],
                wk8_d[j, :, :, g * P:(g + 1) * P],
            )
    for j in range(KP):
        nc.gpsimd.dma_start(xt8_b0[j][:, :, 512:T_], xt8_d[0, j, :, :, 512:T_])
    for k in range(KT):
        t = xt_pool.tile([P, P], BF16, tag="xt")
        nc.gpsimd.dma_start(t[:], xtb_d[0, k * P:(k + 1) * P, :])
        xt_b0.append(t)
    u_sb = const.tile([P, P], BF16, tag="u")
    nc.sync.dma_start(u_sb[:], u_d[:])
    ones_sb = const.tile([1, P], F32R, tag="ones")
    nc.sync.dma_start(ones_sb[:], ones_d[:])
    id_sb = const.tile([P, P], BF16, tag="ident")
    nc.sync.dma_start(id_sb[:], id_d[:])
    for k in range(KT):
        t2 = const.tile([P, HD], BF16, tag=f"wv{k}")
        nc.sync.dma_start(t2[:], wv_d[k * P:(k + 1) * P, :])
        wv_sb.append(t2)
    for j in range(KP):
        t5 = const.tile([P, 2, HD], FP8, tag=f"wv8{j}")
        nc.sync.dma_start(t5[:], wv8_d[j, :, :, :])
        wv8_sb.append(t5)

    def load_x(b):
        xt = []
        for k in range(KT):
            t = xt_pool.tile([P, P], BF16, tag="xt")
            nc.sync.dma_start(t[:], xtb_d[b, k * P:(k + 1) * P, :])
            xt.append(t)
        xt8 = []
        for j in range(KP):
            t = xt8_pool.tile([P, 2, T_], FP8, tag="xt8")
            nc.sync.dma_start(t[:], xt8_d[b, j, :, :, :])
            xt8.append(t)
        return xt, xt8

    def k_group_closures(xt8_b, sT):
        # ---- K path: s^T[h, t], fully fp8 DoubleRow. Returned as one
        # closure per psum-group so the caller can interleave the NEXT
        # batch's K stream into the CURRENT batch's c-loop: dense
        # independent K work keeps PE warm (HAM re-throttles across
        # >3.4us idle gaps, halving the PE clock for ~4us). ----
        holder = {}
        groups = []
        for tc5 in range(TC5):
            for g in range(NG):
                def emit(tc5=tc5, g=g):
                    if g == 0:
                        ps_new = pa_pool.tile([H_, 512], F32, tag="a")
                        holder[tc5] = ps_new
                    ps_s = holder[tc5]
                    pk = pa_pool.tile([P, 512], F32, tag="a")
                    for j in range(KP):
                        nc.tensor.matmul(
                            pk[:],
                            wk8_sb[j][:, :, g * P:(g + 1) * P],
                            xt8_b[j][:, :, tc5 * 512:(tc5 + 1) * 512],
                            start=(j == 0),
                            stop=(j == KP - 1),
                            perf_mode=DR,
                        )
                    # silu straight to fp8 in DoubleRow pair layout: g pairs
                    # (2p, 2p+1) share one [P, 2, 512] tile, so s^T is 4 DR
                    # matmuls per 512-chunk instead of 8 f32r ones.
                    pair, sub = g // 2, g % 2
                    if sub == 0:
                        ksil8_new = ksil_pool.tile([P, 2, 512], FP8, tag="ksil")
                        holder["k", tc5, pair] = ksil8_new
                    ksil8 = holder["k", tc5, pair]
                    nc.scalar.activation(ksil8[:, sub, :], pk[:], AF.Silu)
                    if sub == 1:
                        nc.tensor.matmul(
                            ps_s[:], qb8_sb[pair][:], ksil8[:],
                            start=(pair == 0), stop=(pair == KP - 1),
                            perf_mode=DR,
                        )
                    if g == NG - 1:
                        nc.scalar.copy(sT[:, tc5 * 512:(tc5 + 1) * 512], ps_s[:])
                groups.append(emit)
        return groups

    def den_chain_closures(sT):
        # exp -> den scans -> per-chunk transposes/recips -> rden row
        # rotation, as closures to interleave into the previous batch's
        # c-loop. e_c[c] is [t, h] bf16; rs_all holds 1/den with rows
        # rotated by +1 (row 0 = t=chunk end) to match the rotated cumsum.
        eT = st_pool.tile([H_, T_], BF16, tag="et")
        denT = st_pool.tile([H_, T_], BF16, tag="dt")
        rc_all = rden_pool.tile([P, NT * H_], F32, tag="rc")
        rs_all = rden_pool.tile([P, NT * H_], F32, tag="rs")
        e_c = [None] * NT

        def exp_half(i):
            nc.scalar.activation(
                eT[:, i * 512:(i + 1) * 512], sT[:, i * 512:(i + 1) * 512], AF.Exp
            )

        def scan_half(i):
            init = 0.0 if i == 0 else denT[:, i * 512 - 1:i * 512]
            nc.vector.tensor_tensor_scan(
                denT[:, i * 512:(i + 1) * 512],
                eT[:, i * 512:(i + 1) * 512],
                eT[:, i * 512:(i + 1) * 512],
                init,
                op0=mybir.AluOpType.add, op1=mybir.AluOpType.bypass,
            )

        def tr(c):
            # PE transposes: cheap (~110ns) and NOT on a DMA ring — the
            # xbar DMA transpose alternative measured 1.2us/op of sync-ring
            # time right in the boundary-critical window (total +21us).
            pt_e = pa_pool.tile([P, H_], BF16, tag="a")
            nc.tensor.transpose(pt_e[:], eT[:, c * P:(c + 1) * P], id_sb[:H_, :H_])
            ec = epc_pool.tile([P, H_], BF16, tag="epc")
            nc.vector.tensor_copy(ec[:], pt_e[:])
            e_c[c] = ec
            pt_d = pa_pool.tile([P, H_], BF16, tag="a")
            nc.tensor.transpose(pt_d[:], denT[:, c * P:(c + 1) * P], id_sb[:H_, :H_])
            nc.vector.reciprocal(rc_all[:, c * H_:(c + 1) * H_], pt_d[:])

        def rot(i):
            lo, hi = i * 4 * H_, (i + 1) * 4 * H_
            nc.gpsimd.dma_start(rs_all[0:1, lo:hi], rc_all[P - 1:P, lo:hi])
            nc.gpsimd.dma_start(rs_all[1:P, lo:hi], rc_all[0:P - 1, lo:hi])

        cl = [lambda: exp_half(0), lambda: scan_half(0),
              lambda: exp_half(1), lambda: scan_half(1)]
        for c in range(NT // 2):
            cl.append(lambda c=c: tr(c))
        cl.append(lambda: rot(0))
        for c in range(NT // 2, NT):
            cl.append(lambda c=c: tr(c))
        cl.append(lambda: rot(1))
        # order: [exp0, scan0, exp1, scan1, tr0..3, rot0, tr4..7, rot1]
        return cl, e_c, rs_all

    def assemble_pending(kg, chain):
        # interleave so each piece is emitted right after its deps:
        # exp0/scan0 after the 8 tc5=0 groups, exp1/scan1 after tc5=1,
        # transposes/rots last. (Emitting the exps back-to-back saves ACT
        # table swaps but serializes an ~8us PE-idle chain at the batch-0
        # boundary, re-throttling HAM — measured 33us slower overall.)
        return (kg[0:8] + chain[0:2] + kg[8:16] + chain[2:4] + chain[4:])

    # batch 0's K + den chain run upfront (nothing earlier to interleave
    # into). x loads run TWO batches ahead so boundary-filling closures
    # never wait on DMA.
    xt_cur, xt8_cur = xt_b0, xt8_b0
    xq = [load_x(1)] if B_loc > 1 else []
    sT_cur = st_pool.tile([H_, T_], F32, tag="st")
    kg0 = k_group_closures(xt8_cur, sT_cur)
    chain0, ec_cur, rs_cur = den_chain_closures(sT_cur)

    # V projection + silu; emitted PREFETCH chunks ahead inside the c-loop
    # to keep PE busy while the chunk chain resolves.
    PREFETCH = 2

    def v_proj(xt, xt8, c):
        vsil = vsil_pool.tile([P, HD], BF16, tag="vsil")
        for nb in range(NB):
            pv = pv_pool.tile([P, 512], F32, tag="v")
            if c == 0:
                for k in range(KT):
                    nc.tensor.matmul(
                        pv[:],
                        xt[k][:, 0:P],
                        wv_sb[k][:, nb * 512:(nb + 1) * 512],
                        start=(k == 0),
                        stop=(k == KT - 1),
                    )
            else:
                for j in range(KP):
                    nc.tensor.matmul(
                        pv[:],
                        xt8[j][:, :, c * P:(c + 1) * P],
                        wv8_sb[j][:, :, nb * 512:(nb + 1) * 512],
                        start=(j == 0),
                        stop=(j == KP - 1),
                        perf_mode=DR,
                    )
            nc.scalar.activation(vsil[:, nb * 512:(nb + 1) * 512], pv[:], AF.Silu)
        return vsil

    # batch 0 upfront: V chunks 0/1 interleave into the K stream so PE has
    # work while the exp/scan chain resolves, and tr4-7/rot1 spill into the
    # first two c-loop chunks (they're only needed from chunk 4).
    vq_next = []
    for emit in (kg0[0:8]
                 + [lambda: vq_next.append(v_proj(xt_b0, xt8_b0, 0))]
                 + chain0[0:2] + kg0[8:16]
                 + [lambda: vq_next.append(v_proj(xt_b0, xt8_b0, 1))]
                 + chain0[2:4] + chain0[4:9]):
        emit()
    spill = chain0[9:14]
    scuts = [0, 3, 5, 5, 5, 5, 5, 5, 5]

    for b in range(B_loc):
        xt, xt8, e_c, rs_all = xt_cur, xt8_cur, ec_cur, rs_cur
        if b + 2 < B_loc:
            xq.append(load_x(b + 2))
        if b + 1 < B_loc:
            xt_cur, xt8_cur = xq.pop(0)
            sT_cur = st_pool.tile([H_, T_], F32, tag="st")
            kg = k_group_closures(xt8_cur, sT_cur)
            chain, ec_cur, rs_cur = den_chain_closures(sT_cur)
            pending = assemble_pending(kg, chain)
            # prime the NEXT batch's first V chunks at the end of this
            # batch's c-loop so its chunk 0 never waits on ACT/PE at the
            # boundary.
            vq_coming = []
            pending = pending + [
                lambda xt_=xt_cur, x8_=xt8_cur: vq_coming.append(
                    v_proj(xt_, x8_, 0)),
                lambda xt_=xt_cur, x8_=xt8_cur: vq_coming.append(
                    v_proj(xt_, x8_, 1)),
            ]
            spill_next = []
            sched = [3, 6, 9, 13, 17, 20, 24, 29, 32]
        else:
            pending = []
            vq_coming = []
            spill_next = []
            sched = [0] * 9

        vsil_q = vq_next

        o_stage = o_pool.tile([P, NT * D_], F32, tag="o")

        for emit in pending[0:3]:
            emit()

        # ---- V path with rotated running num cumsum ----
        # Ushift columns: out row 0 = chunk total (+carry) = inclusive prefix
        # at t=P-1; row m>=1 = inclusive prefix at t=m-1. Row 0 is the legal
        # (base-partition-0) carry source for the next chunk's K=1 broadcast
        # matmul. The host un-rolls the rotation.
        prev_cum = None
        for c in range(NT):
            vsil = vsil_q[c]
            if c + PREFETCH < NT:
                vsil_q.append(v_proj(xt, xt8, c + PREFETCH))

            ve = ve_pool.tile([P, HD], BF16, tag="ve")
            e_bc = e_c[c][:].unsqueeze(1).broadcast_to((P, D_, H_))
            nc.vector.tensor_mul(
                ve[:].rearrange("p (d h) -> p d h", h=H_),
                vsil[:].rearrange("p (d h) -> p d h", h=H_),
                e_bc,
            )

            cum = cum_pool.tile([P, HD], F32R, tag="cum")
            pcs = []
            for nb in range(NB):
                pc = pc_pool.tile([P, 512], F32, tag="c")
                nc.tensor.matmul(
                    pc[:], u_sb[:], ve[:, nb * 512:(nb + 1) * 512],
                    start=True, stop=(c == 0),
                )
                pcs.append(pc)
            if c > 0:
                for nb in range(NB):
                    nc.tensor.matmul(
                        pcs[nb][:], ones_sb[:],
                        prev_cum[0:1, nb * 512:(nb + 1) * 512],
                        start=False, stop=True,
                    )
            # psum->sbuf copies split across ACT and DVE to balance engines
            nc.scalar.copy(cum[:, 0:512], pcs[0][:])
            nc.vector.tensor_copy(cum[:, 512:HD], pcs[1][:])
            prev_cum = cum

            # prod = num * (1/den); head-sum is a unit-stride reduce thanks
            # to the (d, h) column order. prod stays f32: TENSOR_REDUCE
            # accumulates at input precision (bf16 prod measured 1.7e-2 rel
            # err vs 2e-2 gate) and bf16 gave no DVE speedup here anyway.
            prod = prod_pool.tile([P, HD], F32, tag="prod")
            r_bc = rs_all[:, c * H_:(c + 1) * H_].unsqueeze(1).broadcast_to(
                (P, D_, H_)
            )
            nc.vector.tensor_mul(
                prod[:].rearrange("p (d h) -> p d h", h=H_),
                cum[:].bitcast(F32).rearrange("p (d h) -> p d h", h=H_),
                r_bc,
            )
            nc.vector.reduce_sum(
                o_stage[:, c * D_:(c + 1) * D_],
                prod[:].rearrange("p (d h) -> p d h", h=H_),
                axis=mybir.AxisListType.X,
            )
            if c == NT - 3:
                nc.sync.dma_start(
                    out_d[b, :, 0:NT - 2, :],
                    o_stage[:, 0:(NT - 2) * D_].rearrange("p (c d) -> p c d", d=D_),
                )

            # interleave the next batch's K groups + den chain to keep the
            # PE stream dense across the batch boundary.
            for emit in pending[sched[c]:sched[c + 1]]:
                emit()
            for emit in spill[scuts[c]:scuts[c + 1]]:
                emit()
            if b == B_loc - 1:
                # warm-keepers: the last c-loop has no successor K stream;
                # HAM re-throttles PE to half clock across its lean stretch
                # (measured ~35us at K=4/8). Dead DR matmuls into the idle
                # pa bank keep the clock at 2.4GHz for the real work.
                for _ in range(2 if c < 5 else 4):
                    dmy = pa_pool.tile([P, 512], F32, tag="a")
                    nc.tensor.matmul(
                        dmy[:], wk8_sb[0][:, :, 0:P], xt8[0][:, :, 0:512],
                        start=True, stop=True, perf_mode=DR,
                    )

        nc.sync.dma_start(
            out_d[b, :, NT - 2:NT, :],
            o_stage[:, (NT - 2) * D_:NT * D_].rearrange("p (c d) -> p c d", d=D_),
        )
        vq_next = vq_coming
        spill = spill_next
        scuts = [0] * 9


_NC_CACHE = []


def _build_nc():
    if _NC_CACHE:
        return _NC_CACHE[0]
    _apply_tile_patches()
    nc = bass.Bass(trn_type="TRN2", target_bir_lowering=False, debug=False)
    with tile.TileContext(nc) as tc:
        with ExitStack() as ctx:
            _build(nc, tc, ctx)
    _NC_CACHE.append(nc)
    return nc


def _fp8(a):
    return np.asarray(np.clip(a, -240.0, 240.0), dtype=ml_dtypes.float8_e4m3fn)


def _pair_pack(w):
    # [I, F] -> [KP, P, 2, F] with [j, p, s, f] = w[j*256 + s*128 + p, f]
    F = w.shape[1]
    return np.ascontiguousarray(
        w.reshape(I_DIM // 256, 2, P, F).transpose(0, 2, 1, 3)
    )


def _host_prep(x_shard, shared):
    xt = np.ascontiguousarray(x_shard.transpose(0, 2, 1))  # [B_loc, I, T] f32
    m = dict(shared)
    m["xtb"] = xt[:, :, 0:P].astype(ml_dtypes.bfloat16)
    xt8 = _fp8(xt)  # [B_loc, I, T]
    m["xt8"] = np.ascontiguousarray(
        xt8.reshape(B_LOC, I_DIM // 256, 2, P, T).transpose(0, 1, 3, 2, 4)
    )
    return m


def kernel(x, kv_kernel, q_kernel):
    x = np.asarray(x, dtype=np.float32)
    kv_kernel = np.asarray(kv_kernel, dtype=np.float32)
    q_kernel = np.asarray(q_kernel, dtype=np.float32)
    HD = H * D

    wk = np.ascontiguousarray(kv_kernel[..., 0].reshape(I_DIM, HD))
    # V columns permuted to (d, h) so the on-chip head-sum is unit-stride
    wv = np.ascontiguousarray(
        kv_kernel[..., 1].transpose(0, 2, 1).reshape(I_DIM, HD)
    )
    qb = np.zeros((HD, H), dtype=np.float32)
    for h in range(H):
        qb[h * D:(h + 1) * D, h] = q_kernel[h]
    u = np.triu(np.ones((P, P), dtype=np.float32), k=1)
    u[:, 0] = 1.0
    shared = {
        "wk8": _pair_pack(_fp8(wk)),
        "wv": wv.astype(ml_dtypes.bfloat16),
        "wv8": _pair_pack(_fp8(wv)),
        "qb8": _pair_pack(_fp8(qb)),
        "u": u.astype(ml_dtypes.bfloat16),
        "ones": np.ones((1, P), dtype=np.float32),
        "ident": np.eye(P, dtype=ml_dtypes.bfloat16),
    }

    nc = _build_nc()
    in_maps = [
        _host_prep(x[c * B_LOC:(c + 1) * B_LOC], shared)
        for c in range(N_CORES)
    ]
    res = bass_utils.run_bass_kernel_spmd(nc, in_maps, core_ids=list(range(N_CORES)))
    # out is [B_loc, P(rotated rows), NT, D]: row m = t=c*128+m-1, row 0 =
    # t=c*128+127. Un-roll and reshape to [B, T, D].
    out = np.concatenate([r["out"] for r in res.results], axis=0)
    out = np.roll(out, -1, axis=1).transpose(0, 2, 1, 3).reshape(B, T, D)
    return out.astype(np.float32)


# revision 45
# speedup vs baseline: 1.1324x; 1.1324x over previous
"""AttentionRNNCell (streaming-softmax attention RNN) for 8 TRN2 NeuronCores.

kernel(x, kv_kernel, q_kernel) -> [B, T, D] float32

Math per (batch, head): kv = silu(x @ kv_kernel); s_t = <q_h, k_t>;
out_t = sum_h cumsum_t(v * e^s) / cumsum_t(e^s)   (unstabilized streaming
softmax — safe for this data distribution; |s| stays < ~8).

Strategy (data-parallel over batch, 4 batches/core):
  - Projections run in fp8e4m3 DoubleRow mode (2 fp8 weights/cell, 256-deep
    contraction): K path fully fp8 (its quantization noise only perturbs
    softmax weights, which streaming averaging washes out), V path fp8 for
    t>=128 with the first chunk bf16 (at small t, h_t averages few v's, so
    V-path fp8 noise passes straight through).
  - K^T = Wk^T @ x^T on PE in [head*dim, t] layout (f32 psum), silu on ACT
    straight out of PSUM, s^T = Qblock^T @ silu(K^T) on PE, exp on ACT,
    den = cumsum via DVE tensor_tensor_scan along t, PE-transposes bring
    e/1/den back to [t, h] layout.  The whole exp/scan/transpose/recip
    chain for batch b+1 is emitted as closures interleaved into batch b's
    chunk loop (together with b+1's K groups) so the PE never stalls at a
    batch boundary (HAM re-throttles after ~3.4us of idle).
  - V columns are permuted to (d, h) order on the host so the final
    head-sum reduce is unit-stride and the e/rden broadcasts are
    inner-contiguous (DVE 2x mode for the 16-bit ve multiply).
  - V = x^T.T @ Wv in [t, (d h)] layout, silu to bf16, ve = v*e (DVE bf16
    broadcast multiply), cumsum over t via a column-rotated triangular-ones
    matmul (output row 0 = running total -> legal K=1 carry-broadcast
    source for the next chunk), psum->sbuf cum copies split ACT/DVE,
    out = sum_h num * (1/den) via DVE multiply (bf16 out) + contiguous
    reduce into a per-batch staging tile, stored with 2 large DMAs.
    Rotation is undone on the host (free np.roll).
"""

import numpy as np
from contextlib import ExitStack

import ml_dtypes

import bass_rust
import concourse.bass as bass
import concourse.mybir as mybir
import concourse.tile as tile
from concourse import bass_utils

AF = mybir.ActivationFunctionType
BF16 = mybir.dt.bfloat16
FP8 = mybir.dt.float8e4
F32 = mybir.dt.float32
F32R = mybir.dt.float32r
DR = mybir.MatmulPerfMode.DoubleRow

P = 128
N_CORES = 8
B, T, I_DIM, H, D = 32, 1024, 1024, 16, 64
B_LOC = B // N_CORES


# ---------------------------------------------------------------------------
# TileContext patches: the walrus build in this container supports only ONE
# semaphore wait per instruction. (1) split the end-of-context drain's waits
# across several drains; (2) hoist extra scheduler-attached waits onto
# InstNoOp carriers just before the instruction on the same engine.
# ---------------------------------------------------------------------------

def _split_waits(self, inst):
    si = inst.sync_info
    if (
        si is not None
        and si.on_wait
        and len(si.on_wait) > 1
        and inst.engine != mybir.EngineType.Unassigned
    ):
        waits = list(si.on_wait)
        sem_waits = [w for w in waits if w.sync_type == "semaphore"]
        other = [w for w in waits if w.sync_type != "semaphore"]
        hoist = sem_waits[:-1] if sem_waits else []
        keep = sem_waits[-1:] + other if sem_waits else other
        if hoist:
            for w in hoist:
                nop = mybir.InstNoOp(
                    name=self.nc.get_next_instruction_name(),
                    sync_info=mybir.SyncInfo(on_wait=[w], on_update=[]),
                    bass_nofuse=True,
                    engine=inst.engine,
                )
                self.nc.register_instruction(nop, overwrite=True)
                self.nc.cur_bb.bb.add_instruction(nop)
            inst.sync_info = mybir.SyncInfo(
                on_wait=keep, on_update=list(si.on_update or [])
            )


def _patched_add_instruction(self, inst):
    _split_waits(self, inst)
    self.nc.register_instruction(inst, overwrite=True)
    self.nc.cur_bb.bb.add_instruction(inst)


def _patched_drain_and_barrier(self, tick_clock, wait_clock):
    nc = self.nc
    drain_inst = nc.sync.drain()
    wait_clock.add_sem_waits(
        drain_inst.ins, bass_rust.ScopedClock({None: tick_clock.global_clock})
    )
    si = drain_inst.ins.sync_info
    waits = list(si.on_wait) if si is not None and si.on_wait else []
    if len(waits) > 1:
        upds = list(si.on_update) if si.on_update else []
        drain_inst.ins.sync_info = bass_rust.SyncInfo(
            on_wait=[waits[0]], on_update=upds
        )
        for w in waits[1:]:
            extra = nc.sync.drain()
            extra.ins.sync_info = bass_rust.SyncInfo(on_wait=[w], on_update=[])

    nc.all_engine_barrier()
    assert self.sems is not None
    popped = nc._tile_sem_poison_stack.pop()
    assert popped is self._sem_poison
    nc.clear_and_free_semaphores(list(self.sems.allocated().values()))
    nc.all_engine_barrier()


def _apply_tile_patches():
    tile.TileContext._add_instruction = _patched_add_instruction
    tile.TileContext._drain_and_barrier = _patched_drain_and_barrier


# ---------------------------------------------------------------------------
# Kernel builder
# ---------------------------------------------------------------------------

def _build(nc, tc, ctx):
    B_loc, T_, I, H_, D_ = B_LOC, T, I_DIM, H, D
    HD = H_ * D_
    NT = T_ // P
    KT = I // P
    KP = KT // 2  # fp8 pair-blocks of 256 along I
    NG = HD // P
    NB = HD // 512
    TC5 = T_ // 512

    # bf16 inputs cover only the accuracy-critical early timesteps of the
    # V path (t<128); the K path runs fully in fp8 (its quantization noise
    # only perturbs softmax weights, which the streaming average washes out).
    xtb_d = nc.dram_tensor("xtb", [B_loc, I, P], BF16, kind="ExternalInput").ap()
    # fp8 x^T packed as I-block pairs: [b, j, p, s, t] = x^T[b, j*256+s*128+p, t]
    xt8_d = nc.dram_tensor("xt8", [B_loc, KP, P, 2, T_], FP8, kind="ExternalInput").ap()
    wk8_d = nc.dram_tensor("wk8", [KP, P, 2, HD], FP8, kind="ExternalInput").ap()
    wv_d = nc.dram_tensor("wv", [I, HD], BF16, kind="ExternalInput").ap()
    wv8_d = nc.dram_tensor("wv8", [KP, P, 2, HD], FP8, kind="ExternalInput").ap()
    qb8_d = nc.dram_tensor("qb8", [KP, P, 2, H_], FP8, kind="ExternalInput").ap()
    u_d = nc.dram_tensor("u", [P, P], BF16, kind="ExternalInput").ap()
    ones_d = nc.dram_tensor("ones", [1, P], F32R, kind="ExternalInput").ap()
    id_d = nc.dram_tensor("ident", [P, P], BF16, kind="ExternalInput").ap()
    # rotated rows: row m holds t=c*128+m-1 (row 0: t=c*128+127); host un-rolls.
    out_d = nc.dram_tensor("out", [B_loc, P, NT, D_], F32, kind="ExternalOutput").ap()

    const = ctx.enter_context(tc.tile_pool(name="const", bufs=1))
    xt_pool = ctx.enter_context(tc.tile_pool(name="xt", bufs=3 * KT))
    xt8_pool = ctx.enter_context(tc.tile_pool(name="xt8", bufs=3 * KP))
    ksil_pool = ctx.enter_context(tc.tile_pool(name="ksil", bufs=2))
    st_pool = ctx.enter_context(tc.tile_pool(name="st", bufs=6))
    epc_pool = ctx.enter_context(tc.tile_pool(name="epc", bufs=3 * NT))
    rden_pool = ctx.enter_context(tc.tile_pool(name="rden", bufs=4))
    vsil_pool = ctx.enter_context(tc.tile_pool(name="vsil", bufs=4))
    ve_pool = ctx.enter_context(tc.tile_pool(name="ve", bufs=2))
    cum_pool = ctx.enter_context(tc.tile_pool(name="cum", bufs=3))
    prod_pool = ctx.enter_context(tc.tile_pool(name="prod", bufs=2))
    o_pool = ctx.enter_context(tc.tile_pool(name="o", bufs=2))

    # PSUM: 8 banks. pa (3, shared tag) = K-path accumulators + transposes;
    # pv/pc 2 each -> 7 banks. (8/8 deadlocks the slot scheduler.)
    pa_pool = ctx.enter_context(tc.tile_pool(name="pa", bufs=3, space="PSUM"))
    pv_pool = ctx.enter_context(tc.tile_pool(name="pv", bufs=2, space="PSUM"))
    pc_pool = ctx.enter_context(tc.tile_pool(name="pc", bufs=2, space="PSUM"))

    # ---- weights/constants. Two DMA rings: sync carries weights, gpsimd
    # carries batch-0 activations, so the first K group's operands land in
    # parallel instead of serially on one ring. qb[0] goes first so the
    # act-table warm-up (below) starts immediately. ----
    qb8_sb = []
    for j in range(KP):
        t3 = const.tile([P, 2, H_], FP8, tag=f"qb8{j}")
        nc.sync.dma_start(t3[:], qb8_d[j, :, :, :])
        qb8_sb.append(t3)
    # dummy activation: pulls the one-time Silu act-table load (~2.7us,
    # measured as a PE stall before the first K silu) into the startup DMA
    # window where PE is idle anyway.
    warm = const.tile([P, H_], F32, tag="warm")
    nc.scalar.activation(warm[:], qb8_sb[0][:, 0, :], AF.Silu)

    wk8_sb, wv_sb, wv8_sb = [], [], []
    xt_b0, xt8_b0 = [], []
    for j in range(KP):
        t4 = const.tile([P, 2, HD], FP8, tag=f"wk8{j}")
        wk8_sb.append(t4)
        t = xt8_pool.tile([P, 2, T_], FP8, tag="xt8")
        nc.gpsimd.dma_start(t[:, :, 0:512], xt8_d[0, j, :, :, 0:512])
        xt8_b0.append(t)
    # wk8 lands g-major so the first K psum-group's weights (g=0 slices of
    # all 4 j) arrive in ~4 small DMAs instead of after the full 1MB load.
    for g in range(NG):
        for j in range(KP):
            nc.sync.dma_start(
                wk8_sb[j][:, :, g * P:(g + 1) * P],
                wk8_d[j, :, :, g * P:(g + 1) * P],
            )
    for j in range(KP):
        nc.gpsimd.dma_start(xt8_b0[j][:, :, 512:T_], xt8_d[0, j, :, :, 512:T_])
    for k in range(KT):
        t = xt_pool.tile([P, P], BF16, tag="xt")
        nc.gpsimd.dma_start(t[:], xtb_d[0, k * P:(k + 1) * P, :])
        xt_b0.append(t)
    u_sb = const.tile([P, P], BF16, tag="u")
    nc.sync.dma_start(u_sb[:], u_d[:])
    ones_sb = const.tile([1, P], F32R, tag="ones")
    nc.sync.dma_start(ones_sb[:], ones_d[:])
    id_sb = const.tile([P, P], BF16, tag="ident")
    nc.sync.dma_start(id_sb[:], id_d[:])
    for k in range(KT):
        t2 = const.tile([P, HD], BF16, tag=f"wv{k}")
        nc.sync.dma_start(t2[:], wv_d[k * P:(k + 1) * P, :])
        wv_sb.append(t2)
    for j in range(KP):
        t5 = const.tile([P, 2, HD], FP8, tag=f"wv8{j}")
        nc.sync.dma_start(t5[:], wv8_d[j, :, :, :])
        wv8_sb.append(t5)

    def load_x(b):
        xt = []
        for k in range(KT):
            t = xt_pool.tile([P, P], BF16, tag="xt")
            nc.sync.dma_start(t[:], xtb_d[b, k * P:(k + 1) * P, :])
            xt.append(t)
        xt8 = []
        for j in range(KP):
            t = xt8_pool.tile([P, 2, T_], FP8, tag="xt8")
            nc.sync.dma_start(t[:], xt8_d[b, j, :, :, :])
            xt8.append(t)
        return xt, xt8

    def k_group_closures(xt8_b, sT):
        # ---- K path: s^T[h, t], fully fp8 DoubleRow. Returned as one
        # closure per psum-group so the caller can interleave the NEXT
        # batch's K stream into the CURRENT batch's c-loop: dense
        # independent K work keeps PE warm (HAM re-throttles across
        # >3.4us idle gaps, halving the PE clock for ~4us). ----
        holder = {}
        groups = []
        for tc5 in range(TC5):
            for g in range(NG):
                def emit(tc5=tc5, g=g):
                    if g == 0:
                        ps_new = pa_pool.tile([H_, 512], F32, tag="a")
                        holder[tc5] = ps_new
                    ps_s = holder[tc5]
                    pk = pa_pool.tile([P, 512], F32, tag="a")
                    for j in range(KP):
                        nc.tensor.matmul(
                            pk[:],
                            wk8_sb[j][:, :, g * P:(g + 1) * P],
                            xt8_b[j][:, :, tc5 * 512:(tc5 + 1) * 512],
                            start=(j == 0),
                            stop=(j == KP - 1),
                            perf_mode=DR,
                        )
                    # silu straight to fp8 in DoubleRow pair layout: g pairs
                    # (2p, 2p+1) share one [P, 2, 512] tile, so s^T is 4 DR
                    # matmuls per 512-chunk instead of 8 f32r ones.
                    pair, sub = g // 2, g % 2
                    if sub == 0:
                        ksil8_new = ksil_pool.tile([P, 2, 512], FP8, tag="ksil")
                        holder["k", tc5, pair] = ksil8_new
                    ksil8 = holder["k", tc5, pair]
                    nc.scalar.activation(ksil8[:, sub, :], pk[:], AF.Silu)
                    if sub == 1:
                        nc.tensor.matmul(
                            ps_s[:], qb8_sb[pair][:], ksil8[:],
                            start=(pair == 0), stop=(pair == KP - 1),
                            perf_mode=DR,
                        )
                    if g == NG - 1:
                        nc.scalar.copy(sT[:, tc5 * 512:(tc5 + 1) * 512], ps_s[:])
                groups.append(emit)
        return groups

    def den_chain_closures(sT):
        # exp -> den scans -> per-chunk transposes/recips -> rden row
        # rotation, as closures to interleave into the previous batch's
        # c-loop. e_c[c] is [t, h] bf16; rs_all holds 1/den with rows
        # rotated by +1 (row 0 = t=chunk end) to match the rotated cumsum.
        eT = st_pool.tile([H_, T_], BF16, tag="et")
        denT = st_pool.tile([H_, T_], BF16, tag="dt")
        rc_all = rden_pool.tile([P, NT * H_], F32, tag="rc")
        rs_all = rden_pool.tile([P, NT * H_], F32, tag="rs")
        e_c = [None] * NT

        def exp_half(i):
            nc.scalar.activation(
                eT[:, i * 512:(i + 1) * 512], sT[:, i * 512:(i + 1) * 512], AF.Exp
            )

        def scan_half(i):
            init = 0.0 if i == 0 else denT[:, i * 512 - 1:i * 512]
            nc.vector.tensor_tensor_scan(
                denT[:, i * 512:(i + 1) * 512],
                eT[:, i * 512:(i + 1) * 512],
                eT[:, i * 512:(i + 1) * 512],
                init,
                op0=mybir.AluOpType.add, op1=mybir.AluOpType.bypass,
            )

        def tr(c):
            # PE transposes: cheap (~110ns) and NOT on a DMA ring — the
            # xbar DMA transpose alternative measured 1.2us/op of sync-ring
            # time right in the boundary-critical window (total +21us).
            pt_e = pa_pool.tile([P, H_], BF16, tag="a")
            nc.tensor.transpose(pt_e[:], eT[:, c * P:(c + 1) * P], id_sb[:H_, :H_])
            ec = epc_pool.tile([P, H_], BF16, tag="epc")
            nc.vector.tensor_copy(ec[:], pt_e[:])
            e_c[c] = ec
            pt_d = pa_pool.tile([P, H_], BF16, tag="a")
            nc.tensor.transpose(pt_d[:], denT[:, c * P:(c + 1) * P], id_sb[:H_, :H_])
            nc.vector.reciprocal(rc_all[:, c * H_:(c + 1) * H_], pt_d[:])

        def rot(i):
            lo, hi = i * 4 * H_, (i + 1) * 4 * H_
            nc.gpsimd.dma_start(rs_all[0:1, lo:hi], rc_all[P - 1:P, lo:hi])
            nc.gpsimd.dma_start(rs_all[1:P, lo:hi], rc_all[0:P - 1, lo:hi])

        cl = [lambda: exp_half(0), lambda: scan_half(0),
              lambda: exp_half(1), lambda: scan_half(1)]
        for c in range(NT // 2):
            cl.append(lambda c=c: tr(c))
        cl.append(lambda: rot(0))
        for c in range(NT // 2, NT):
            cl.append(lambda c=c: tr(c))
        cl.append(lambda: rot(1))
        # order: [exp0, scan0, exp1, scan1, tr0..3, rot0, tr4..7, rot1]
        return cl, e_c, rs_all

    def assemble_pending(kg, chain):
        # interleave so each piece is emitted right after its deps:
        # exp0/scan0 after the 8 tc5=0 groups, exp1/scan1 after tc5=1,
        # transposes/rots last. (Emitting the exps back-to-back saves ACT
        # table swaps but serializes an ~8us PE-idle chain at the batch-0
        # boundary, re-throttling HAM — measured 33us slower overall.)
        return (kg[0:8] + chain[0:2] + kg[8:16] + chain[2:4] + chain[4:])

    # batch 0's K + den chain run upfront (nothing earlier to interleave
    # into). x loads run TWO batches ahead so boundary-filling closures
    # never wait on DMA.
    xt_cur, xt8_cur = xt_b0, xt8_b0
    xq = [load_x(1)] if B_loc > 1 else []
    sT_cur = st_pool.tile([H_, T_], F32, tag="st")
    kg0 = k_group_closures(xt8_cur, sT_cur)
    chain0, ec_cur, rs_cur = den_chain_closures(sT_cur)

    # V projection + silu; emitted PREFETCH chunks ahead inside the c-loop
    # to keep PE busy while the chunk chain resolves.
    PREFETCH = 2

    def v_proj(xt, xt8, c):
        vsil = vsil_pool.tile([P, HD], BF16, tag="vsil")
        for nb in range(NB):
            pv = pv_pool.tile([P, 512], F32, tag="v")
            if c == 0:
                for k in range(KT):
                    nc.tensor.matmul(
                        pv[:],
                        xt[k][:, 0:P],
                        wv_sb[k][:, nb * 512:(nb + 1) * 512],
                        start=(k == 0),
                        stop=(k == KT - 1),
                    )
            else:
                for j in range(KP):
                    nc.tensor.matmul(
                        pv[:],
                        xt8[j][:, :, c * P:(c + 1) * P],
                        wv8_sb[j][:, :, nb * 512:(nb + 1) * 512],
                        start=(j == 0),
                        stop=(j == KP - 1),
                        perf_mode=DR,
                    )
            nc.scalar.activation(vsil[:, nb * 512:(nb + 1) * 512], pv[:], AF.Silu)
        return vsil

    # batch 0 upfront: V chunks 0/1 interleave into the K stream so PE has
    # work while the exp/scan chain resolves, and tr4-7/rot1 spill into the
    # first two c-loop chunks (they're only needed from chunk 4).
    # NOTE: transposes must NOT overlap a live K-group stream — both use
    # the 3-slot pa psum ring and the in-order PE queue blocks behind a
    # starved tile allocation (measured +38us when spilled into the c-loop).
    vq_next = []
    for emit in (kg0[0:8]
                 + [lambda: vq_next.append(v_proj(xt_b0, xt8_b0, 0))]
                 + chain0[0:2] + kg0[8:16]
                 + [lambda: vq_next.append(v_proj(xt_b0, xt8_b0, 1))]
                 + chain0[2:4] + chain0[4:14]):
        emit()
    spill = []
    scuts = [0] * 9

    for b in range(B_loc):
        xt, xt8, e_c, rs_all = xt_cur, xt8_cur, ec_cur, rs_cur
        if b + 2 < B_loc:
            xq.append(load_x(b + 2))
        if b + 1 < B_loc:
            xt_cur, xt8_cur = xq.pop(0)
            sT_cur = st_pool.tile([H_, T_], F32, tag="st")
            kg = k_group_closures(xt8_cur, sT_cur)
            chain, ec_cur, rs_cur = den_chain_closures(sT_cur)
            pending = assemble_pending(kg, chain)
            # prime the NEXT batch's first V chunks at the end of this
            # batch's c-loop so its chunk 0 never waits on ACT/PE at the
            # boundary.
            vq_coming = []
            vp0 = lambda xt_=xt_cur, x8_=xt8_cur: vq_coming.append(
                v_proj(xt_, x8_, 0))
            vp1 = lambda xt_=xt_cur, x8_=xt8_cur: vq_coming.append(
                v_proj(xt_, x8_, 1))
            # vp0/vp1 sit between the transpose groups so the next batch's
            # first vsil tiles are silu'd ~2 chunks before its c-loop
            # starts (priming at c7 left a ~2us silu->ve->U chain exposed
            # at each boundary).
            pending = (pending[0:25] + [vp0] + pending[25:29]
                       + [vp1] + pending[29:30])
            spill_next = []
            sched = [3, 6, 9, 13, 17, 20, 24, 28, 32]
        else:
            pending = []
            vq_coming = []
            spill_next = []
            sched = [0] * 9

        vsil_q = vq_next

        o_stage = o_pool.tile([P, NT * D_], F32, tag="o")

        for emit in pending[0:3]:
            emit()

        # ---- V path with rotated running num cumsum ----
        # Ushift columns: out row 0 = chunk total (+carry) = inclusive prefix
        # at t=P-1; row m>=1 = inclusive prefix at t=m-1. Row 0 is the legal
        # (base-partition-0) carry source for the next chunk's K=1 broadcast
        # matmul. The host un-rolls the rotation.
        prev_cum = None
        for c in range(NT):
            vsil = vsil_q[c]
            if c + PREFETCH < NT:
                vsil_q.append(v_proj(xt, xt8, c + PREFETCH))

            ve = ve_pool.tile([P, HD], BF16, tag="ve")
            e_bc = e_c[c][:].unsqueeze(1).broadcast_to((P, D_, H_))
            nc.vector.tensor_mul(
                ve[:].rearrange("p (d h) -> p d h", h=H_),
                vsil[:].rearrange("p (d h) -> p d h", h=H_),
                e_bc,
            )

            cum = cum_pool.tile([P, HD], F32R, tag="cum")
            pcs = []
            for nb in range(NB):
                pc = pc_pool.tile([P, 512], F32, tag="c")
                nc.tensor.matmul(
                    pc[:], u_sb[:], ve[:, nb * 512:(nb + 1) * 512],
                    start=True, stop=(c == 0),
                )
                pcs.append(pc)
            if c > 0:
                for nb in range(NB):
                    nc.tensor.matmul(
                        pcs[nb][:], ones_sb[:],
                        prev_cum[0:1, nb * 512:(nb + 1) * 512],
                        start=False, stop=True,
                    )
            # psum->sbuf copies split across ACT and DVE to balance engines
            nc.scalar.copy(cum[:, 0:512], pcs[0][:])
            nc.vector.tensor_copy(cum[:, 512:HD], pcs[1][:])
            prev_cum = cum

            # prod = num * (1/den); head-sum is a unit-stride reduce thanks
            # to the (d, h) column order. prod stays f32: TENSOR_REDUCE
            # accumulates at input precision (bf16 prod measured 1.7e-2 rel
            # err vs 2e-2 gate) and bf16 gave no DVE speedup here anyway.
            prod = prod_pool.tile([P, HD], F32, tag="prod")
            r_bc = rs_all[:, c * H_:(c + 1) * H_].unsqueeze(1).broadcast_to(
                (P, D_, H_)
            )
            nc.vector.tensor_mul(
                prod[:].rearrange("p (d h) -> p d h", h=H_),
                cum[:].bitcast(F32).rearrange("p (d h) -> p d h", h=H_),
                r_bc,
            )
            nc.vector.reduce_sum(
                o_stage[:, c * D_:(c + 1) * D_],
                prod[:].rearrange("p (d h) -> p d h", h=H_),
                axis=mybir.AxisListType.X,
            )
            if c == NT - 3:
                nc.sync.dma_start(
                    out_d[b, :, 0:NT - 2, :],
                    o_stage[:, 0:(NT - 2) * D_].rearrange("p (c d) -> p c d", d=D_),
                )

            # interleave the next batch's K groups + den chain to keep the
            # PE stream dense across the batch boundary.
            for emit in pending[sched[c]:sched[c + 1]]:
                emit()
            for emit in spill[scuts[c]:scuts[c + 1]]:
                emit()
            if b == B_loc - 1:
                # warm-keepers: the last c-loop has no successor K stream;
                # HAM re-throttles PE to half clock across its lean stretch
                # (measured ~35us at K=4/8). Dead DR matmuls into the idle
                # pa bank keep the clock at 2.4GHz for the real work.
                for _ in range(2 if c < 5 else 4):
                    dmy = pa_pool.tile([P, 512], F32, tag="a")
                    nc.tensor.matmul(
                        dmy[:], wk8_sb[0][:, :, 0:P], xt8[0][:, :, 0:512],
                        start=True, stop=True, perf_mode=DR,
                    )

        nc.sync.dma_start(
            out_d[b, :, NT - 2:NT, :],
            o_stage[:, (NT - 2) * D_:NT * D_].rearrange("p (c d) -> p c d", d=D_),
        )
        vq_next = vq_coming
        spill = spill_next
        scuts = [0] * 9


_NC_CACHE = []


def _build_nc():
    if _NC_CACHE:
        return _NC_CACHE[0]
    _apply_tile_patches()
    nc = bass.Bass(trn_type="TRN2", target_bir_lowering=False, debug=False)
    with tile.TileContext(nc) as tc:
        with ExitStack() as ctx:
            _build(nc, tc, ctx)
    _NC_CACHE.append(nc)
    return nc


def _fp8(a):
    return np.asarray(np.clip(a, -240.0, 240.0), dtype=ml_dtypes.float8_e4m3fn)


def _pair_pack(w):
    # [I, F] -> [KP, P, 2, F] with [j, p, s, f] = w[j*256 + s*128 + p, f]
    F = w.shape[1]
    return np.ascontiguousarray(
        w.reshape(I_DIM // 256, 2, P, F).transpose(0, 2, 1, 3)
    )


def _host_prep(x_shard, shared):
    xt = np.ascontiguousarray(x_shard.transpose(0, 2, 1))  # [B_loc, I, T] f32
    m = dict(shared)
    m["xtb"] = xt[:, :, 0:P].astype(ml_dtypes.bfloat16)
    xt8 = _fp8(xt)  # [B_loc, I, T]
    m["xt8"] = np.ascontiguousarray(
        xt8.reshape(B_LOC, I_DIM // 256, 2, P, T).transpose(0, 1, 3, 2, 4)
    )
    return m


def kernel(x, kv_kernel, q_kernel):
    x = np.asarray(x, dtype=np.float32)
    kv_kernel = np.asarray(kv_kernel, dtype=np.float32)
    q_kernel = np.asarray(q_kernel, dtype=np.float32)
    HD = H * D

    wk = np.ascontiguousarray(kv_kernel[..., 0].reshape(I_DIM, HD))
    # V columns permuted to (d, h) so the on-chip head-sum is unit-stride
    wv = np.ascontiguousarray(
        kv_kernel[..., 1].transpose(0, 2, 1).reshape(I_DIM, HD)
    )
    qb = np.zeros((HD, H), dtype=np.float32)
    for h in range(H):
        qb[h * D:(h + 1) * D, h] = q_kernel[h]
    u = np.triu(np.ones((P, P), dtype=np.float32), k=1)
    u[:, 0] = 1.0
    shared = {
        "wk8": _pair_pack(_fp8(wk)),
        "wv": wv.astype(ml_dtypes.bfloat16),
        "wv8": _pair_pack(_fp8(wv)),
        "qb8": _pair_pack(_fp8(qb)),
        "u": u.astype(ml_dtypes.bfloat16),
        "ones": np.ones((1, P), dtype=np.float32),
        "ident": np.eye(P, dtype=ml_dtypes.bfloat16),
    }

    nc = _build_nc()
    in_maps = [
        _host_prep(x[c * B_LOC:(c + 1) * B_LOC], shared)
        for c in range(N_CORES)
    ]
    res = bass_utils.run_bass_kernel_spmd(nc, in_maps, core_ids=list(range(N_CORES)))
    # out is [B_loc, P(rotated rows), NT, D]: row m = t=c*128+m-1, row 0 =
    # t=c*128+127. Un-roll and reshape to [B, T, D].
    out = np.concatenate([r["out"] for r in res.results], axis=0)
    out = np.roll(out, -1, axis=1).transpose(0, 2, 1, 3).reshape(B, T, D)
    return out.astype(np.float32)
